# revision 7
# baseline (speedup 1.0000x reference)
"""Distributed attention kernel for 8 TRN2 NeuronCores.

Sharding: core c -> (batch b = c // 4, head-group g = c % 4).
Each core computes, for its batch element, 4 of the 16 heads end-to-end
(QKV projection, rotary, attention, output projection), producing a
partial output for the full [S, D] result. The host sums the 4 group
partials per batch element (the "all-reduce after wo" done at unshard).

All layouts are pre-arranged on the host so the device does zero
transposes:
  - xT    [D, S]   : x[b].T                       (rhs for qk / lhsT for v)
  - wqk   [D, 1024]: q,k weight rows (rotary-pair-permuted) transposed
  - wv    [D, 512] : v weight rows transposed
  - wo    [512, D] : wo columns for this group, transposed
  - tabc  [128, S] : cos table doubled across both partition halves
  - tabs  [128, S] : sin table, rows 0:64 = -sin, 64:128 = +sin
                     (1/sqrt(hd) folded into wq)

Rotary trick: q/k weight rows are permuted per head so dims [0:64] are
the even (real) rotary components and [64:128] the odd (imag) ones.
Then rotary is plain elementwise math on partition halves (the sign of
the sin term is baked into the table, so the combine is one full-width
add). Scores are invariant to this permutation since q and k share it.

Attention is computed transposed (scores^T[j, i]) so the softmax
numerator AND attn@v need no transposes.

Schedule (the point of this version): the kernel is PE-bound overall
(~331us of matmul per core) but the attention inner loop is paced by
the ACT engine's exp. Mitigations:
  1. Score matmuls write PAIRS of PSUM banks ([128, 2, 512] f32) and a
     single exp instruction covers both, amortizing ACT's ~352-cycle
     per-instruction overhead (exp: 184us -> 147us).
  2. The wo output projection is interleaved INTO the attention stream
     (one 4-matmul wo group every other score pair, consuming the
     previous i-tile's finished attention rows), so the PE stays dense
     while ACT paces the exp.
  3. x streams through SBUF in quarters (2-buf ring, whose slots also
     host wo), and qk/v projections are st-ordered to consume quarters,
     which frees SBUF and lets the next rep's DMA overlap this rep's
     attention tail.
  4. The softmax denominator sums exp tiles on the DVE as a bf16
     pair/quad tree (2x mode) + 3 f32 combines, finished by one f32r
     ones-matmul per i-tile (cross-partition reduce whose psum rows all
     equal l -- a free partition broadcast); the division is applied to
     the raw attn@v output.
"""

import numpy as np
import ml_dtypes

import concourse.tile as tile
from concourse import bacc, mybir
from concourse.bass_utils import run_bass_kernel_spmd

B, S, D = 2, 2048, 2048
NH, HD = 16, 128
N_CORES = 8
GROUPS = 4
LH = NH // GROUPS  # 4 local heads
EQK = 2 * LH * HD  # 1024 (q chunks then k chunks)
EV = LH * HD  # 512
P = 128
DC = D // P  # 16 contraction chunks over d
SC = S // P  # 16 chunks over s
F = 512  # matmul moving free dim (1 PSUM bank of f32)
NT = S // F  # 4

CDT = mybir.dt.bfloat16
NP_CDT = ml_dtypes.bfloat16
F32 = mybir.dt.float32
NP_OUT = NP_CDT  # device out dtype (partials; host upcasts + sums)


def build_graph(num_devices: int = N_CORES, reps: int = 1):
    """reps > 1 replicates the whole computation (timing instrumentation)."""
    nc = bacc.Bacc(
        "TRN2", target_bir_lowering=False, debug=False, num_devices=num_devices
    )
    xT = nc.dram_tensor("xT", [D, S], CDT, kind="ExternalInput").ap()
    wqk = nc.dram_tensor("wqk", [D, EQK], CDT, kind="ExternalInput").ap()
    wv = nc.dram_tensor("wv", [D, EV], CDT, kind="ExternalInput").ap()
    wo = nc.dram_tensor("wo", [EV, D], CDT, kind="ExternalInput").ap()
    tabc = nc.dram_tensor("tabc", [P, S], CDT, kind="ExternalInput").ap()
    tabs = nc.dram_tensor("tabs", [P, S], CDT, kind="ExternalInput").ap()
    out = nc.dram_tensor("out", [S, D], CDT, kind="ExternalOutput").ap()

    xT_r = xT.rearrange("(c p) s -> p c s", p=P)  # [128, 16, 2048]
    wqk_r = wqk.rearrange("(c p) e -> p c e", p=P)  # [128, 16, 1024]
    wv_r = wv.rearrange("(c p) e -> p c e", p=P)  # [128, 16, 512]
    wo_r = wo.rearrange("(c p) o -> p c o", p=P)  # [128, 4, 2048]
    out_r = out.rearrange("(c p) o -> c p o", p=P)  # [16, 128, 2048]

    Exp = mybir.ActivationFunctionType.Exp

    with tile.TileContext(nc) as tc:
        with (
            # xq slots hold x quarters during projection; the ring also
            # hosts wo (same 16KB/partition size) during attention.
            tc.tile_pool(name="xq", bufs=2) as xqp,
            tc.tile_pool(name="wqkp", bufs=1) as wqkp,
            tc.tile_pool(name="wvp", bufs=1) as wvp,  # wv slot, reused for attn
            tc.tile_pool(name="data", bufs=1) as data,
            tc.tile_pool(name="tmp", bufs=2) as tmpp,
            tc.tile_pool(name="expp", bufs=4) as expp,
            tc.tile_pool(name="tree", bufs=2) as treep,
            tc.tile_pool(name="small", bufs=2) as small,
            tc.tile_pool(name="ostage", bufs=6) as ostagep,
            tc.tile_pool(name="mm", bufs=2, space="PSUM") as mm_pool,
            tc.tile_pool(name="acc", bufs=2, space="PSUM") as acc_pool,
            tc.tile_pool(name="lsum", bufs=1, space="PSUM") as l_pool,
            tc.tile_pool(name="pw", bufs=1, space="PSUM") as pw_pool,
        ):
          # Constants loaded once (not per rep).
          tabc_sb = data.tile([P, S], CDT, tag="tabc")
          nc.sync.dma_start(tabc_sb[:], tabc)
          tabs_sb = data.tile([P, S], CDT, tag="tabs")
          nc.sync.dma_start(tabs_sb[:], tabs)
          # f32 ones; bitcast to float32r at the reduce matmul
          # (1 cyc/row at N=512, ~1e-4 matmul precision)
          ones_f32 = data.tile([P, P], F32, tag="ones32")
          nc.vector.memset(ones_f32[:], 1.0)
          ones_fr = data.tile([P, P], mybir.dt.float32r, tag="ones")
          nc.vector.tensor_copy(out=ones_fr[:], in_=ones_f32[:])

          for _rep in range(reps):
            # ---------------- loads ----------------
            # The first projection group accumulates c = 0..15 in order, so
            # interleave wqk's first ec-pair chunk with x quarter 0 per
            # c-chunk: the first matmul is ready after ~200KB, and the rest
            # of the accumulation paces with DMA arrival.
            wqk_sb = wqkp.tile([P, DC, EQK], CDT, tag="wqk")

            def load_xq(st):
                t = xqp.tile([P, DC, F], CDT, tag="xq", name=f"xq{st}")
                for c in range(DC):
                    nc.sync.dma_start(t[:, c, :], xT_r[:, c, st * F : (st + 1) * F])
                return t

            xq0 = xqp.tile([P, DC, F], CDT, tag="xq", name="xq0")
            for c in range(DC):
                nc.sync.dma_start(
                    wqk_sb[:, c, 0 : 2 * P], wqk_r[:, c, 0 : 2 * P]
                )
                nc.sync.dma_start(xq0[:, c, :], xT_r[:, c, 0:F])
            xq_next = xq0
            wv_sb = wvp.tile([P, DC, EV], CDT, tag="wv")
            for c in range(DC):
                nc.sync.dma_start(wv_sb[:, c, :], wv_r[:, c, :])
            for pe in range(1, 4):
                nc.sync.dma_start(
                    wqk_sb[:, 0:DC, pe * 2 * P : (pe + 1) * 2 * P],
                    wqk_r[:, 0:DC, pe * 2 * P : (pe + 1) * 2 * P],
                )

            rot_sb = data.tile([P, 2 * LH, S], CDT, tag="rot")
            v_sb = data.tile([P, SC, EV], CDT, tag="v")

            # ---------------- qkv projection + rotary (st-streamed) ------
            # qkT[e, s] = sum_d wqk[d, e] * xT[d, s]; rotary into rot_sb.
            # v[s, e]   = sum_d xT[d, s] * wv[d, e].
            # Each st quarter of x is fully consumed before the next, so x
            # only ever occupies two 16KB quarter slots.
            for st in range(NT):
                xq = xq_next
                if st + 1 < NT:
                    xq_next = load_xq(st + 1)
                sl = slice(st * F, (st + 1) * F)
                for pp in range(4):  # ec pairs: (q0,q1),(q2,q3),(k0,k1),(k2,k3)
                    ps = mm_pool.tile([P, 2, F], F32, tag="mm", name="psqk")
                    for e in range(2):
                        ec = 2 * pp + e
                        for c in range(DC):
                            nc.tensor.matmul(
                                ps[:, e, :],
                                lhsT=wqk_sb[:, c, ec * P : (ec + 1) * P],
                                rhs=xq[:, c, :],
                                start=(c == 0),
                                stop=(c == DC - 1),
                            )
                    # Stage psum -> bf16 SBUF: qs straight on the DVE, qsw
                    # with partition halves swapped on the (otherwise idle)
                    # scalar engine, reading PSUM directly.
                    # partitions 0:64 = even (re), 64:128 = odd (im)
                    qs = tmpp.tile([P, 2, F], CDT, tag="qs")
                    nc.vector.tensor_copy(out=qs[:], in_=ps[:])
                    qsw = tmpp.tile([P, 2, F], CDT, tag="qsw")
                    nc.scalar.copy(out=qsw[0:64], in_=ps[64:128])
                    nc.scalar.copy(out=qsw[64:128], in_=ps[0:64])
                    t1 = tmpp.tile([P, 2, F], CDT, tag="t1")
                    t2 = tmpp.tile([P, 2, F], CDT, tag="t2")
                    for e in range(2):
                        nc.vector.tensor_mul(t1[:, e, :], qs[:, e, :], tabc_sb[:, sl])
                        nc.vector.tensor_mul(t2[:, e, :], qsw[:, e, :], tabs_sb[:, sl])
                    # tabs carries the sign split (-sin top / +sin bottom),
                    # so re' and im' are one full-width add.
                    nc.vector.tensor_add(
                        rot_sb[:, 2 * pp : 2 * pp + 2, sl], t1[:], t2[:]
                    )
                for vp in range(2):  # v pairs: local sc chunks (0,1),(2,3)
                    ps = mm_pool.tile([P, 2, F], F32, tag="mm", name="psv")
                    for e in range(2):
                        scl = 2 * vp + e
                        for c in range(DC):
                            nc.tensor.matmul(
                                ps[:, e, :],
                                lhsT=xq[:, c, scl * P : (scl + 1) * P],
                                rhs=wv_sb[:, c, :],
                                start=(c == 0),
                                stop=(c == DC - 1),
                            )
                    sc0 = 4 * st + 2 * vp
                    nc.vector.tensor_copy(out=v_sb[:, sc0 : sc0 + 2, :], in_=ps[:])

            # wo loads into a free x-quarter slot; attn reuses the wv slot.
            wo_sb = xqp.tile([P, LH, D], CDT, tag="xq", name="wo_sb")
            for c in range(LH):
                nc.sync.dma_start(wo_sb[:, c, :], wo_r[:, c, :])
            attn_sb = wvp.tile([P, LH, S], CDT, tag="wv", name="attn_sb")

            # ---------------- attention + interleaved wo ----------------
            # Per score pair: 2 score matmuls into a 2-bank psum pair, one
            # exp over both banks (ACT), 2 attn@v accumulating matmuls.
            # ACT paces this at ~1147ns/pair vs 852ns of PE work, so every
            # other pair the PE also runs one wo group (4 matmuls) from the
            # previous i-tile's attention rows.
            def emit_wo_group(sc, ot):
                osl = slice(ot * F, (ot + 1) * F)
                pw = pw_pool.tile([P, F], F32, tag="pw", name="pw")
                for hc in range(LH):
                    nc.tensor.matmul(
                        pw[:],
                        lhsT=attn_sb[:, hc, sc * P : (sc + 1) * P],
                        rhs=wo_sb[:, hc, osl],
                        start=(hc == 0),
                        stop=(hc == LH - 1),
                    )
                # Stage on the DVE, store on the sync ring. Keep the scalar
                # engine's queue exp-only: its strict FIFO means anything
                # here that waits on the wo matmuls would stall later exps
                # behind the PE and cascade.
                ost = ostagep.tile([P, F], CDT, tag="ostage")
                nc.vector.tensor_copy(out=ost[:], in_=pw[:])
                nc.sync.dma_start(out_r[sc, :, osl], ost[:])

            for it in range(NT):
                isl = slice(it * F, (it + 1) * F)
                fillers = (
                    [
                        (sc, ot)
                        for sc in range(4 * (it - 1), 4 * it)
                        for ot in range(NT)
                    ]
                    if it > 0
                    else []
                )
                fi = 0
                for h in range(LH):
                    po = acc_pool.tile([P, F], F32, tag="acc", name="po")
                    accl = None
                    # Quad blocks: 2 score pairs -> exps -> 1 wo filler group
                    # -> 4 av matmuls. The wo group sits BETWEEN the scores
                    # and the avs so its ~850ns of independent PE work covers
                    # the exp latency (av j depends on exp j); without it the
                    # PE would stall ~400ns per av group. it0 has no wo
                    # filler, so it uses single-512 exps to halve the exp
                    # latency instead.
                    for qq in range(SC // 4):
                        pss = []
                        ets = []
                        for half in range(2):
                            ps = mm_pool.tile([P, 2, F], F32, tag="mm", name="pss")
                            for e in range(2):
                                jc = 4 * qq + 2 * half + e
                                # scores^T[j, i] = sum_hd k[hd, j] * q[hd, i]
                                nc.tensor.matmul(
                                    ps[:, e, :],
                                    lhsT=rot_sb[:, LH + h, jc * P : (jc + 1) * P],
                                    rhs=rot_sb[:, h, isl],
                                    start=True,
                                    stop=True,
                                )
                            pss.append(ps)
                        for half in range(2):
                            et = expp.tile([P, 2, F], CDT, tag="exp")
                            if it == 0:
                                for e in range(2):
                                    nc.scalar.activation(
                                        out=et[:, e, :],
                                        in_=pss[half][:, e, :],
                                        func=Exp,
                                    )
                            else:
                                nc.scalar.activation(
                                    out=et[:], in_=pss[half][:], func=Exp
                                )
                            ets.append(et)
                        if fi < len(fillers):
                            emit_wo_group(*fillers[fi])
                            fi += 1
                        for half in range(2):
                            for e in range(2):
                                jc = 4 * qq + 2 * half + e
                                nc.tensor.matmul(
                                    po[:],
                                    lhsT=v_sb[:, jc, h * P : (h + 1) * P],
                                    rhs=ets[half][:, e, :],
                                    start=(jc == 0),
                                    stop=(jc == SC - 1),
                                )
                        # Denominator tree on the DVE: bf16 pair/quad adds
                        # in the 2x mode, then f32 combines into accl.
                        pr0 = treep.tile([P, F], CDT, tag="pr")
                        nc.vector.tensor_add(pr0[:], ets[0][:, 0, :], ets[0][:, 1, :])
                        pr1 = treep.tile([P, F], CDT, tag="pr2")
                        nc.vector.tensor_add(pr1[:], ets[1][:, 0, :], ets[1][:, 1, :])
                        qd = treep.tile([P, F], CDT, tag="qd")
                        nc.vector.tensor_add(qd[:], pr0[:], pr1[:])
                        if qq == 0:
                            accl = small.tile([P, F], mybir.dt.float32r, tag="accl")
                            prev_qd = qd
                        elif qq == 1:
                            nc.vector.tensor_add(accl[:], prev_qd[:], qd[:])
                        else:
                            nc.vector.tensor_add(accl[:], accl[:], qd[:])
                    pl = l_pool.tile([P, F], F32, tag="lsum", name="pl")
                    nc.tensor.matmul(
                        pl[:],
                        lhsT=ones_fr[:],
                        rhs=accl[:],
                        start=True,
                        stop=True,
                    )
                    # pl rows are all equal (ones lhsT) -> reciprocal is
                    # already "broadcast" across partitions.
                    rl128 = small.tile([P, F], F32, tag="recip128")
                    nc.vector.reciprocal_approx_fast(rl128[:], pl[:])
                    nc.vector.tensor_mul(attn_sb[:, h, isl], po[:], rl128[:])
                assert fi == len(fillers)

            # wo tail: the last i-tile's output rows.
            for sc in range(4 * (NT - 1), 4 * NT):
                for ot in range(NT):
                    emit_wo_group(sc, ot)

    nc.compile()
    return nc


def shard_inputs(x, freqs_cis, wqkv, wo):
    """Produce the 8 per-core input maps (host-side layout prep)."""
    x = np.asarray(x, dtype=np.float32)
    freqs_cis = np.asarray(freqs_cis, dtype=np.float32)
    wqkv = np.asarray(wqkv, dtype=np.float32)
    wo = np.asarray(wo, dtype=np.float32)

    perm = np.concatenate([np.arange(0, HD, 2), np.arange(1, HD, 2)])  # even|odd
    cos = freqs_cis[:, :, 0].T  # [64, S]
    sin = freqs_cis[:, :, 1].T
    scale = 1.0 / np.sqrt(HD)  # folded into wq rows below
    tabc = np.concatenate([cos, cos], axis=0)  # [128, S]
    tabs = np.concatenate([-sin, sin], axis=0)  # sign baked in

    tabc = np.ascontiguousarray(tabc.astype(NP_CDT))
    tabs = np.ascontiguousarray(tabs.astype(NP_CDT))

    in_maps = []
    for c in range(N_CORES):
        b, g = divmod(c, GROUPS)
        heads = range(g * LH, (g + 1) * LH)
        wq_rows = np.concatenate(
            [wqkv[h * HD : (h + 1) * HD][perm] for h in heads], axis=0
        ) * scale  # [512, D]; 1/sqrt(hd) folded in
        wk_rows = np.concatenate(
            [wqkv[D + h * HD : D + (h + 1) * HD][perm] for h in heads], axis=0
        )
        wv_rows = np.concatenate(
            [wqkv[2 * D + h * HD : 2 * D + (h + 1) * HD] for h in heads], axis=0
        )
        wqk_l = np.concatenate([wq_rows, wk_rows], axis=0).T  # [D, 1024]
        wv_l = wv_rows.T  # [D, 512]
        din = np.concatenate([np.arange(h * HD, (h + 1) * HD) for h in heads])
        wo_l = wo[:, din].T  # [512, D]
        in_maps.append(
            {
                "xT": np.ascontiguousarray(x[b].T.astype(NP_CDT)),
                "wqk": np.ascontiguousarray(wqk_l.astype(NP_CDT)),
                "wv": np.ascontiguousarray(wv_l.astype(NP_CDT)),
                "wo": np.ascontiguousarray(wo_l.astype(NP_CDT)),
                "tabc": tabc,
                "tabs": tabs,
            }
        )
    return in_maps


def unshard_outputs(results):
    out = np.zeros((B, S, D), dtype=np.float32)
    for c in range(N_CORES):
        b = c // GROUPS
        out[b] += results[c]["out"].astype(np.float32)
    return out


_GRAPH_CACHE = {}


def kernel(x, freqs_cis, wqkv, wo):
    if "nc" not in _GRAPH_CACHE:
        _GRAPH_CACHE["nc"] = build_graph()
    nc = _GRAPH_CACHE["nc"]
    in_maps = shard_inputs(x, freqs_cis, wqkv, wo)
    res = run_bass_kernel_spmd(nc, in_maps, core_ids=list(range(N_CORES)))
    return unshard_outputs(res.results)


# revision 9
# speedup vs baseline: 1.6538x; 1.6538x over previous
"""Distributed attention kernel for 8 TRN2 NeuronCores.

Sharding: core c -> (batch b = c // 4, head-group g = c % 4).
Each core computes, for its batch element, 4 of the 16 heads end-to-end
(QKV projection, rotary, attention, output projection), producing a
partial output for the full [S, D] result. The host sums the 4 group
partials per batch element (the "all-reduce after wo" done at unshard).

All layouts are pre-arranged on the host so the device does zero
transposes:
  - xT    [D, S]   : x[b].T                       (rhs for qk / lhsT for v)
  - wqk   [D, 1024]: q,k weight rows (rotary-pair-permuted) transposed
  - wv    [D, 512] : v weight rows transposed
  - wo    [512, D] : wo columns for this group, transposed
  - tabc  [128, S] : cos table doubled across both partition halves
  - tabs  [128, S] : sin table, rows 0:64 = -sin, 64:128 = +sin
                     (1/sqrt(hd) folded into wq)

Rotary trick: q/k weight rows are permuted per head so dims [0:64] are
the even (real) rotary components and [64:128] the odd (imag) ones.
Then rotary is plain elementwise math on partition halves (the sign of
the sin term is baked into the table, so the combine is one full-width
add). Scores are invariant to this permutation since q and k share it.

Attention is computed transposed (scores^T[j, i]) so the softmax
numerator AND attn@v need no transposes.

Schedule (the point of this version): the kernel is PE-bound overall
(~331us of matmul per core) but the attention inner loop is paced by
the ACT engine's exp. Mitigations:
  1. Score matmuls write PAIRS of PSUM banks ([128, 2, 512] f32) and a
     single exp instruction covers both, amortizing ACT's ~352-cycle
     per-instruction overhead (exp: 184us -> 147us).
  2. The wo output projection is interleaved INTO the attention stream
     (one 4-matmul wo group every other score pair, consuming the
     previous i-tile's finished attention rows), so the PE stays dense
     while ACT paces the exp.
  3. x streams through SBUF in quarters (2-buf ring, whose slots also
     host wo), and qk/v projections are st-ordered to consume quarters,
     which frees SBUF and lets the next rep's DMA overlap this rep's
     attention tail.
  4. The softmax denominator sums exp tiles on the DVE as a bf16
     pair/quad tree (2x mode) + 3 f32 combines, finished by one f32r
     ones-matmul per i-tile (cross-partition reduce whose psum rows all
     equal l -- a free partition broadcast); the division is applied to
     the raw attn@v output.
"""

import numpy as np
import ml_dtypes

import concourse.tile as tile
from concourse import bacc, mybir
from concourse.bass_utils import run_bass_kernel_spmd

B, S, D = 2, 2048, 2048
NH, HD = 16, 128
N_CORES = 8
GROUPS = 4
LH = NH // GROUPS  # 4 local heads
EQK = 2 * LH * HD  # 1024 (q chunks then k chunks)
EV = LH * HD  # 512
P = 128
DC = D // P  # 16 contraction chunks over d
SC = S // P  # 16 chunks over s
F = 512  # matmul moving free dim (1 PSUM bank of f32)
NT = S // F  # 4

CDT = mybir.dt.bfloat16
NP_CDT = ml_dtypes.bfloat16
F32 = mybir.dt.float32
NP_OUT = NP_CDT  # device out dtype (partials; host upcasts + sums)


def build_graph(num_devices: int = N_CORES, reps: int = 1):
    """reps > 1 replicates the whole computation (timing instrumentation)."""
    nc = bacc.Bacc(
        "TRN2", target_bir_lowering=False, debug=False, num_devices=num_devices
    )
    xT = nc.dram_tensor("xT", [D, S], CDT, kind="ExternalInput").ap()
    wqk = nc.dram_tensor("wqk", [D, EQK], CDT, kind="ExternalInput").ap()
    wv = nc.dram_tensor("wv", [D, EV], CDT, kind="ExternalInput").ap()
    wo = nc.dram_tensor("wo", [EV, D], CDT, kind="ExternalInput").ap()
    tabc = nc.dram_tensor("tabc", [P, S], CDT, kind="ExternalInput").ap()
    tabs = nc.dram_tensor("tabs", [P, S], CDT, kind="ExternalInput").ap()
    out = nc.dram_tensor("out", [S, D], CDT, kind="ExternalOutput").ap()

    xT_r = xT.rearrange("(c p) s -> p c s", p=P)  # [128, 16, 2048]
    wqk_r = wqk.rearrange("(c p) e -> p c e", p=P)  # [128, 16, 1024]
    wv_r = wv.rearrange("(c p) e -> p c e", p=P)  # [128, 16, 512]
    wo_r = wo.rearrange("(c p) o -> p c o", p=P)  # [128, 4, 2048]
    out_r = out.rearrange("(c p) o -> c p o", p=P)  # [16, 128, 2048]

    Exp = mybir.ActivationFunctionType.Exp

    with tile.TileContext(nc) as tc:
        with (
            # xq slots hold x quarters during projection; the ring also
            # hosts wo (same 16KB/partition size) during attention.
            tc.tile_pool(name="xq", bufs=2) as xqp,
            tc.tile_pool(name="wqkp", bufs=1) as wqkp,
            tc.tile_pool(name="wvp", bufs=1) as wvp,  # wv slot, reused for attn
            tc.tile_pool(name="data", bufs=1) as data,
            tc.tile_pool(name="tmp", bufs=2) as tmpp,
            tc.tile_pool(name="expp", bufs=4) as expp,
            tc.tile_pool(name="tree", bufs=2) as treep,
            tc.tile_pool(name="small", bufs=2) as small,
            tc.tile_pool(name="ostage", bufs=6) as ostagep,
            tc.tile_pool(name="mm", bufs=2, space="PSUM") as mm_pool,
            tc.tile_pool(name="acc", bufs=2, space="PSUM") as acc_pool,
            tc.tile_pool(name="lsum", bufs=1, space="PSUM") as l_pool,
            tc.tile_pool(name="pw", bufs=1, space="PSUM") as pw_pool,
        ):
          # Constants loaded once (not per rep).
          tabc_sb = data.tile([P, S], CDT, tag="tabc")
          nc.sync.dma_start(tabc_sb[:], tabc)
          tabs_sb = data.tile([P, S], CDT, tag="tabs")
          nc.sync.dma_start(tabs_sb[:], tabs)
          # f32 ones; bitcast to float32r at the reduce matmul
          # (1 cyc/row at N=512, ~1e-4 matmul precision)
          ones_f32 = data.tile([P, P], F32, tag="ones32")
          nc.vector.memset(ones_f32[:], 1.0)
          ones_fr = data.tile([P, P], mybir.dt.float32r, tag="ones")
          nc.vector.tensor_copy(out=ones_fr[:], in_=ones_f32[:])

          for _rep in range(reps):
            # ---------------- loads ----------------
            # wqk arrives in ec-pair chunks so the first projection group
            # only waits on ~3MB (first wqk chunk + x quarter 0).
            wqk_sb = wqkp.tile([P, DC, EQK], CDT, tag="wqk")
            nc.sync.dma_start(wqk_sb[:, 0:DC, 0 : 2 * P], wqk_r[:, 0:DC, 0 : 2 * P])

            def load_xq(st):
                t = xqp.tile([P, DC, F], CDT, tag="xq", name=f"xq{st}")
                for c in range(DC):
                    nc.sync.dma_start(t[:, c, :], xT_r[:, c, st * F : (st + 1) * F])
                return t

            xq_next = load_xq(0)
            wv_sb = wvp.tile([P, DC, EV], CDT, tag="wv")
            for c in range(DC):
                nc.sync.dma_start(wv_sb[:, c, :], wv_r[:, c, :])
            for pe in range(1, 4):
                nc.sync.dma_start(
                    wqk_sb[:, 0:DC, pe * 2 * P : (pe + 1) * 2 * P],
                    wqk_r[:, 0:DC, pe * 2 * P : (pe + 1) * 2 * P],
                )

            rot_sb = data.tile([P, 2 * LH, S], CDT, tag="rot")
            v_sb = data.tile([P, SC, EV], CDT, tag="v")

            # ---------------- qkv projection + rotary (st-streamed) ------
            # qkT[e, s] = sum_d wqk[d, e] * xT[d, s]; rotary into rot_sb.
            # v[s, e]   = sum_d xT[d, s] * wv[d, e].
            # Each st quarter of x is fully consumed before the next, so x
            # only ever occupies two 16KB quarter slots.
            for st in range(NT):
                xq = xq_next
                if st + 1 < NT:
                    xq_next = load_xq(st + 1)
                sl = slice(st * F, (st + 1) * F)
                for pp in range(4):  # ec pairs: (q0,q1),(q2,q3),(k0,k1),(k2,k3)
                    ps = mm_pool.tile([P, 2, F], F32, tag="mm", name="psqk")
                    for e in range(2):
                        ec = 2 * pp + e
                        for c in range(DC):
                            nc.tensor.matmul(
                                ps[:, e, :],
                                lhsT=wqk_sb[:, c, ec * P : (ec + 1) * P],
                                rhs=xq[:, c, :],
                                start=(c == 0),
                                stop=(c == DC - 1),
                            )
                    # Stage psum -> bf16 SBUF: qs straight on the DVE, qsw
                    # with partition halves swapped on the (otherwise idle)
                    # scalar engine, reading PSUM directly.
                    # partitions 0:64 = even (re), 64:128 = odd (im)
                    qs = tmpp.tile([P, 2, F], CDT, tag="qs")
                    nc.vector.tensor_copy(out=qs[:], in_=ps[:])
                    qsw = tmpp.tile([P, 2, F], CDT, tag="qsw")
                    nc.scalar.copy(out=qsw[0:64], in_=ps[64:128])
                    nc.scalar.copy(out=qsw[64:128], in_=ps[0:64])
                    t1 = tmpp.tile([P, 2, F], CDT, tag="t1")
                    t2 = tmpp.tile([P, 2, F], CDT, tag="t2")
                    for e in range(2):
                        nc.vector.tensor_mul(t1[:, e, :], qs[:, e, :], tabc_sb[:, sl])
                        nc.vector.tensor_mul(t2[:, e, :], qsw[:, e, :], tabs_sb[:, sl])
                    # tabs carries the sign split (-sin top / +sin bottom),
                    # so re' and im' are one full-width add.
                    nc.vector.tensor_add(
                        rot_sb[:, 2 * pp : 2 * pp + 2, sl], t1[:], t2[:]
                    )
                for vp in range(2):  # v pairs: local sc chunks (0,1),(2,3)
                    ps = mm_pool.tile([P, 2, F], F32, tag="mm", name="psv")
                    for e in range(2):
                        scl = 2 * vp + e
                        for c in range(DC):
                            nc.tensor.matmul(
                                ps[:, e, :],
                                lhsT=xq[:, c, scl * P : (scl + 1) * P],
                                rhs=wv_sb[:, c, :],
                                start=(c == 0),
                                stop=(c == DC - 1),
                            )
                    sc0 = 4 * st + 2 * vp
                    nc.vector.tensor_copy(out=v_sb[:, sc0 : sc0 + 2, :], in_=ps[:])

            # wo loads into a free x-quarter slot; attn reuses the wv slot.
            wo_sb = xqp.tile([P, LH, D], CDT, tag="xq", name="wo_sb")
            for c in range(LH):
                nc.sync.dma_start(wo_sb[:, c, :], wo_r[:, c, :])
            attn_sb = wvp.tile([P, LH, S], CDT, tag="wv", name="attn_sb")

            # ---------------- attention + interleaved wo ----------------
            # Per score pair: 2 score matmuls into a 2-bank psum pair, one
            # exp over both banks (ACT), 2 attn@v accumulating matmuls.
            # ACT paces this at ~1147ns/pair vs 852ns of PE work, so every
            # other pair the PE also runs one wo group (4 matmuls) from the
            # previous i-tile's attention rows.
            def emit_wo_group(sc, ot):
                osl = slice(ot * F, (ot + 1) * F)
                pw = pw_pool.tile([P, F], F32, tag="pw", name="pw")
                for hc in range(LH):
                    nc.tensor.matmul(
                        pw[:],
                        lhsT=attn_sb[:, hc, sc * P : (sc + 1) * P],
                        rhs=wo_sb[:, hc, osl],
                        start=(hc == 0),
                        stop=(hc == LH - 1),
                    )
                ost = ostagep.tile([P, F], CDT, tag="ostage")
                nc.vector.tensor_copy(out=ost[:], in_=pw[:])
                nc.sync.dma_start(out_r[sc, :, osl], ost[:])

            for it in range(NT):
                isl = slice(it * F, (it + 1) * F)
                fillers = (
                    [
                        (sc, ot)
                        for sc in range(4 * (it - 1), 4 * it)
                        for ot in range(NT)
                    ]
                    if it > 0
                    else []
                )
                fi = 0
                pair_idx = 0
                for h in range(LH):
                    po = acc_pool.tile([P, F], F32, tag="acc", name="po")
                    prs = []
                    qds = []
                    accl = None

                    def emit_avs(et, jj):
                        for e in range(2):
                            jc = 2 * jj + e
                            nc.tensor.matmul(
                                po[:],
                                lhsT=v_sb[:, jc, h * P : (h + 1) * P],
                                rhs=et[:, e, :],
                                start=(jc == 0),
                                stop=(jc == SC - 1),
                            )

                    # The av matmuls LAG their pair by 2: av(j) depends on
                    # exp(j), and issued back-to-back the PE would stall on
                    # the exp latency. Two pairs of scores + a wo group in
                    # between (~1.7us of independent PE work) cover it. The
                    # psum pair is freed by the exp itself (avs read the
                    # SBUF et tile), so this costs no extra PSUM banks.
                    pending = []
                    for jj in range(SC // 2):
                        ps = mm_pool.tile([P, 2, F], F32, tag="mm", name="pss")
                        for e in range(2):
                            jc = 2 * jj + e
                            # scores^T[j, i] = sum_hd k[hd, j] * q[hd, i]
                            nc.tensor.matmul(
                                ps[:, e, :],
                                lhsT=rot_sb[:, LH + h, jc * P : (jc + 1) * P],
                                rhs=rot_sb[:, h, isl],
                                start=True,
                                stop=True,
                            )
                        et = expp.tile([P, 2, F], CDT, tag="exp")
                        nc.scalar.activation(out=et[:], in_=ps[:], func=Exp)
                        if pair_idx % 2 == 1 and fi < len(fillers):
                            emit_wo_group(*fillers[fi])
                            fi += 1
                        pair_idx += 1
                        pending.append((et, jj))
                        if len(pending) > 2:
                            emit_avs(*pending.pop(0))
                        # Denominator tree on the DVE: bf16 pair/quad adds
                        # in the 2x mode, then f32 combines into accl.
                        pr = treep.tile([P, F], CDT, tag="pr")
                        nc.vector.tensor_add(pr[:], et[:, 0, :], et[:, 1, :])
                        prs.append(pr)
                        if len(prs) == 2:
                            qd = treep.tile([P, F], CDT, tag="qd")
                            nc.vector.tensor_add(qd[:], prs[0][:], prs[1][:])
                            prs = []
                            qds.append(qd)
                            if len(qds) == 2:
                                if accl is None:
                                    accl = small.tile(
                                        [P, F], mybir.dt.float32r, tag="accl"
                                    )
                                    nc.vector.tensor_add(
                                        accl[:], qds[0][:], qds[1][:]
                                    )
                                else:
                                    nc.vector.tensor_add(
                                        accl[:], accl[:], qds[0][:]
                                    )
                                    nc.vector.tensor_add(
                                        accl[:], accl[:], qds[1][:]
                                    )
                                qds = []
                    pl = l_pool.tile([P, F], F32, tag="lsum", name="pl")
                    nc.tensor.matmul(
                        pl[:],
                        lhsT=ones_fr[:],
                        rhs=accl[:],
                        start=True,
                        stop=True,
                    )
                    # pl rows are all equal (ones lhsT) -> reciprocal is
                    # already "broadcast" across partitions.
                    rl128 = small.tile([P, F], F32, tag="recip128")
                    nc.vector.reciprocal_approx_fast(rl128[:], pl[:])
                    nc.vector.tensor_mul(attn_sb[:, h, isl], po[:], rl128[:])
                assert fi == len(fillers)

            # wo tail: the last i-tile's output rows.
            for sc in range(4 * (NT - 1), 4 * NT):
                for ot in range(NT):
                    emit_wo_group(sc, ot)

    nc.compile()
    return nc


def shard_inputs(x, freqs_cis, wqkv, wo):
    """Produce the 8 per-core input maps (host-side layout prep)."""
    x = np.asarray(x, dtype=np.float32)
    freqs_cis = np.asarray(freqs_cis, dtype=np.float32)
    wqkv = np.asarray(wqkv, dtype=np.float32)
    wo = np.asarray(wo, dtype=np.float32)

    perm = np.concatenate([np.arange(0, HD, 2), np.arange(1, HD, 2)])  # even|odd
    cos = freqs_cis[:, :, 0].T  # [64, S]
    sin = freqs_cis[:, :, 1].T
    scale = 1.0 / np.sqrt(HD)  # folded into wq rows below
    tabc = np.concatenate([cos, cos], axis=0)  # [128, S]
    tabs = np.concatenate([-sin, sin], axis=0)  # sign baked in

    tabc = np.ascontiguousarray(tabc.astype(NP_CDT))
    tabs = np.ascontiguousarray(tabs.astype(NP_CDT))

    in_maps = []
    for c in range(N_CORES):
        b, g = divmod(c, GROUPS)
        heads = range(g * LH, (g + 1) * LH)
        wq_rows = np.concatenate(
            [wqkv[h * HD : (h + 1) * HD][perm] for h in heads], axis=0
        ) * scale  # [512, D]; 1/sqrt(hd) folded in
        wk_rows = np.concatenate(
            [wqkv[D + h * HD : D + (h + 1) * HD][perm] for h in heads], axis=0
        )
        wv_rows = np.concatenate(
            [wqkv[2 * D + h * HD : 2 * D + (h + 1) * HD] for h in heads], axis=0
        )
        wqk_l = np.concatenate([wq_rows, wk_rows], axis=0).T  # [D, 1024]
        wv_l = wv_rows.T  # [D, 512]
        din = np.concatenate([np.arange(h * HD, (h + 1) * HD) for h in heads])
        wo_l = wo[:, din].T  # [512, D]
        in_maps.append(
            {
                "xT": np.ascontiguousarray(x[b].T.astype(NP_CDT)),
                "wqk": np.ascontiguousarray(wqk_l.astype(NP_CDT)),
                "wv": np.ascontiguousarray(wv_l.astype(NP_CDT)),
                "wo": np.ascontiguousarray(wo_l.astype(NP_CDT)),
                "tabc": tabc,
                "tabs": tabs,
            }
        )
    return in_maps


def unshard_outputs(results):
    out = np.zeros((B, S, D), dtype=np.float32)
    for c in range(N_CORES):
        b = c // GROUPS
        out[b] += results[c]["out"].astype(np.float32)
    return out


_GRAPH_CACHE = {}


def kernel(x, freqs_cis, wqkv, wo):
    if "nc" not in _GRAPH_CACHE:
        _GRAPH_CACHE["nc"] = build_graph()
    nc = _GRAPH_CACHE["nc"]
    in_maps = shard_inputs(x, freqs_cis, wqkv, wo)
    res = run_bass_kernel_spmd(nc, in_maps, core_ids=list(range(N_CORES)))
    return unshard_outputs(res.results)


# revision 12
# speedup vs baseline: 1.6877x; 1.0205x over previous
"""Distributed attention kernel for 8 TRN2 NeuronCores.

Sharding: core c -> (batch b = c // 4, head-group g = c % 4).
Each core computes, for its batch element, 4 of the 16 heads end-to-end
(QKV projection, rotary, attention, output projection), producing a
partial output for the full [S, D] result. The host sums the 4 group
partials per batch element (the "all-reduce after wo" done at unshard).

All layouts are pre-arranged on the host so the device does zero
transposes:
  - xT    [D, S]   : x[b].T                       (rhs for qk / lhsT for v)
  - wqk   [D, 1024]: q,k weight rows (rotary-pair-permuted) transposed
  - wv    [D, 512] : v weight rows transposed
  - wo    [512, D] : wo columns for this group, transposed
  - tabc  [128, S] : cos table doubled across both partition halves
  - tabs  [128, S] : sin table, rows 0:64 = -sin, 64:128 = +sin
                     (1/sqrt(hd) folded into wq)

Rotary trick: q/k weight rows are permuted per head so dims [0:64] are
the even (real) rotary components and [64:128] the odd (imag) ones.
Then rotary is plain elementwise math on partition halves (the sign of
the sin term is baked into the table, so the combine is one full-width
add). Scores are invariant to this permutation since q and k share it.

Attention is computed transposed (scores^T[j, i]) so the softmax
numerator AND attn@v need no transposes.

Schedule (the point of this version): the kernel is PE-bound overall
(~331us of matmul per core) but the attention inner loop is paced by
the ACT engine's exp. Mitigations:
  1. Score matmuls write PAIRS of PSUM banks ([128, 2, 512] f32) and a
     single exp instruction covers both, amortizing ACT's ~352-cycle
     per-instruction overhead (exp: 184us -> 147us).
  2. The wo output projection is interleaved INTO the attention stream
     (one 4-matmul wo group every other score pair, consuming the
     previous i-tile's finished attention rows), so the PE stays dense
     while ACT paces the exp.
  3. x streams through SBUF in quarters (2-buf ring, whose slots also
     host wo), and qk/v projections are st-ordered to consume quarters,
     which frees SBUF and lets the next rep's DMA overlap this rep's
     attention tail.
  4. The softmax denominator sums exp tiles on the DVE as a bf16
     pair/quad tree (2x mode) + 3 f32 combines, finished by one f32r
     ones-matmul per i-tile (cross-partition reduce whose psum rows all
     equal l -- a free partition broadcast); the division is applied to
     the raw attn@v output.
"""

import numpy as np
import ml_dtypes

import concourse.tile as tile
from concourse import bacc, mybir
from concourse.bass_utils import run_bass_kernel_spmd

B, S, D = 2, 2048, 2048
NH, HD = 16, 128
N_CORES = 8
GROUPS = 4
LH = NH // GROUPS  # 4 local heads
EQK = 2 * LH * HD  # 1024 (q chunks then k chunks)
EV = LH * HD  # 512
P = 128
DC = D // P  # 16 contraction chunks over d
SC = S // P  # 16 chunks over s
F = 512  # matmul moving free dim (1 PSUM bank of f32)
NT = S // F  # 4

CDT = mybir.dt.bfloat16
NP_CDT = ml_dtypes.bfloat16
F32 = mybir.dt.float32
NP_OUT = NP_CDT  # device out dtype (partials; host upcasts + sums)


def build_graph(num_devices: int = N_CORES, reps: int = 1):
    """reps > 1 replicates the whole computation (timing instrumentation)."""
    nc = bacc.Bacc(
        "TRN2", target_bir_lowering=False, debug=False, num_devices=num_devices
    )
    xT = nc.dram_tensor("xT", [D, S], CDT, kind="ExternalInput").ap()
    wqk = nc.dram_tensor("wqk", [D, EQK], CDT, kind="ExternalInput").ap()
    wv = nc.dram_tensor("wv", [D, EV], CDT, kind="ExternalInput").ap()
    wo = nc.dram_tensor("wo", [EV, D], CDT, kind="ExternalInput").ap()
    tabc = nc.dram_tensor("tabc", [P, S], CDT, kind="ExternalInput").ap()
    tabs = nc.dram_tensor("tabs", [P, S], CDT, kind="ExternalInput").ap()
    out = nc.dram_tensor("out", [S, D], CDT, kind="ExternalOutput").ap()

    xT_r = xT.rearrange("(c p) s -> p c s", p=P)  # [128, 16, 2048]
    wqk_r = wqk.rearrange("(c p) e -> p c e", p=P)  # [128, 16, 1024]
    wv_r = wv.rearrange("(c p) e -> p c e", p=P)  # [128, 16, 512]
    wo_r = wo.rearrange("(c p) o -> p c o", p=P)  # [128, 4, 2048]
    out_r = out.rearrange("(c p) o -> c p o", p=P)  # [16, 128, 2048]

    Exp = mybir.ActivationFunctionType.Exp

    with tile.TileContext(nc) as tc:
        with (
            # xq slots hold x quarters during projection; the ring also
            # hosts wo (same 16KB/partition size) during attention.
            tc.tile_pool(name="xq", bufs=2) as xqp,
            tc.tile_pool(name="wqkp", bufs=1) as wqkp,
            tc.tile_pool(name="wvp", bufs=1) as wvp,  # wv slot, reused for attn
            tc.tile_pool(name="data", bufs=1) as data,
            tc.tile_pool(name="tmp", bufs=2) as tmpp,
            tc.tile_pool(name="expp", bufs=6) as expp,
            tc.tile_pool(name="tree", bufs=2) as treep,
            tc.tile_pool(name="small", bufs=2) as small,
            tc.tile_pool(name="ostage", bufs=6) as ostagep,
            tc.tile_pool(name="mm", bufs=2, space="PSUM") as mm_pool,
            tc.tile_pool(name="acc", bufs=2, space="PSUM") as acc_pool,
            tc.tile_pool(name="lsum", bufs=1, space="PSUM") as l_pool,
            tc.tile_pool(name="pw", bufs=1, space="PSUM") as pw_pool,
        ):
          # Constants loaded once (not per rep).
          tabc_sb = data.tile([P, S], CDT, tag="tabc")
          nc.sync.dma_start(tabc_sb[:], tabc)
          tabs_sb = data.tile([P, S], CDT, tag="tabs")
          nc.sync.dma_start(tabs_sb[:], tabs)
          # f32 ones; bitcast to float32r at the reduce matmul
          # (1 cyc/row at N=512, ~1e-4 matmul precision)
          ones_f32 = data.tile([P, P], F32, tag="ones32")
          nc.vector.memset(ones_f32[:], 1.0)
          ones_fr = data.tile([P, P], mybir.dt.float32r, tag="ones")
          nc.vector.tensor_copy(out=ones_fr[:], in_=ones_f32[:])

          for _rep in range(reps):
            # ---------------- loads ----------------
            # wqk arrives in ec-pair chunks so the first projection group
            # only waits on ~3MB (first wqk chunk + x quarter 0).
            wqk_sb = wqkp.tile([P, DC, EQK], CDT, tag="wqk")
            nc.sync.dma_start(wqk_sb[:, 0:DC, 0 : 2 * P], wqk_r[:, 0:DC, 0 : 2 * P])

            def load_xq(st):
                t = xqp.tile([P, DC, F], CDT, tag="xq", name=f"xq{st}")
                for c in range(DC):
                    nc.sync.dma_start(t[:, c, :], xT_r[:, c, st * F : (st + 1) * F])
                return t

            xq_next = load_xq(0)
            wv_sb = wvp.tile([P, DC, EV], CDT, tag="wv")
            for c in range(DC):
                nc.sync.dma_start(wv_sb[:, c, :], wv_r[:, c, :])
            for pe in range(1, 4):
                nc.sync.dma_start(
                    wqk_sb[:, 0:DC, pe * 2 * P : (pe + 1) * 2 * P],
                    wqk_r[:, 0:DC, pe * 2 * P : (pe + 1) * 2 * P],
                )

            rot_sb = data.tile([P, 2 * LH, S], CDT, tag="rot")
            v_sb = data.tile([P, SC, EV], CDT, tag="v")

            # ---------------- qkv projection + rotary (st-streamed) ------
            # qkT[e, s] = sum_d wqk[d, e] * xT[d, s]; rotary into rot_sb.
            # v[s, e]   = sum_d xT[d, s] * wv[d, e].
            # Each st quarter of x is fully consumed before the next, so x
            # only ever occupies two 16KB quarter slots.
            for st in range(NT):
                xq = xq_next
                if st + 1 < NT:
                    xq_next = load_xq(st + 1)
                sl = slice(st * F, (st + 1) * F)
                for pp in range(4):  # ec pairs: (q0,q1),(q2,q3),(k0,k1),(k2,k3)
                    ps = mm_pool.tile([P, 2, F], F32, tag="mm", name="psqk")
                    for e in range(2):
                        ec = 2 * pp + e
                        for c in range(DC):
                            nc.tensor.matmul(
                                ps[:, e, :],
                                lhsT=wqk_sb[:, c, ec * P : (ec + 1) * P],
                                rhs=xq[:, c, :],
                                start=(c == 0),
                                stop=(c == DC - 1),
                            )
                    # Stage psum -> bf16 SBUF: qs straight on the DVE, qsw
                    # with partition halves swapped on the (otherwise idle)
                    # scalar engine, reading PSUM directly.
                    # partitions 0:64 = even (re), 64:128 = odd (im)
                    qs = tmpp.tile([P, 2, F], CDT, tag="qs")
                    nc.vector.tensor_copy(out=qs[:], in_=ps[:])
                    qsw = tmpp.tile([P, 2, F], CDT, tag="qsw")
                    nc.scalar.copy(out=qsw[0:64], in_=ps[64:128])
                    nc.scalar.copy(out=qsw[64:128], in_=ps[0:64])
                    t1 = tmpp.tile([P, 2, F], CDT, tag="t1")
                    t2 = tmpp.tile([P, 2, F], CDT, tag="t2")
                    for e in range(2):
                        nc.vector.tensor_mul(t1[:, e, :], qs[:, e, :], tabc_sb[:, sl])
                        nc.vector.tensor_mul(t2[:, e, :], qsw[:, e, :], tabs_sb[:, sl])
                    # tabs carries the sign split (-sin top / +sin bottom),
                    # so re' and im' are one full-width add.
                    nc.vector.tensor_add(
                        rot_sb[:, 2 * pp : 2 * pp + 2, sl], t1[:], t2[:]
                    )
                for vp in range(2):  # v pairs: local sc chunks (0,1),(2,3)
                    ps = mm_pool.tile([P, 2, F], F32, tag="mm", name="psv")
                    for e in range(2):
                        scl = 2 * vp + e
                        for c in range(DC):
                            nc.tensor.matmul(
                                ps[:, e, :],
                                lhsT=xq[:, c, scl * P : (scl + 1) * P],
                                rhs=wv_sb[:, c, :],
                                start=(c == 0),
                                stop=(c == DC - 1),
                            )
                    sc0 = 4 * st + 2 * vp
                    nc.vector.tensor_copy(out=v_sb[:, sc0 : sc0 + 2, :], in_=ps[:])

            # wo loads into a free x-quarter slot; attn reuses the wv slot.
            wo_sb = xqp.tile([P, LH, D], CDT, tag="xq", name="wo_sb")
            for c in range(LH):
                nc.sync.dma_start(wo_sb[:, c, :], wo_r[:, c, :])
            attn_sb = wvp.tile([P, LH, S], CDT, tag="wv", name="attn_sb")

            # ---------------- attention + interleaved wo ----------------
            # Per score pair: 2 score matmuls into a 2-bank psum pair, one
            # exp over both banks (ACT), 2 attn@v accumulating matmuls.
            # ACT paces this at ~1147ns/pair vs 852ns of PE work, so every
            # other pair the PE also runs one wo group (4 matmuls) from the
            # previous i-tile's attention rows.
            def emit_wo_group(sc, ot):
                osl = slice(ot * F, (ot + 1) * F)
                pw = pw_pool.tile([P, F], F32, tag="pw", name="pw")
                for hc in range(LH):
                    nc.tensor.matmul(
                        pw[:],
                        lhsT=attn_sb[:, hc, sc * P : (sc + 1) * P],
                        rhs=wo_sb[:, hc, osl],
                        start=(hc == 0),
                        stop=(hc == LH - 1),
                    )
                ost = ostagep.tile([P, F], CDT, tag="ostage")
                nc.vector.tensor_copy(out=ost[:], in_=pw[:])
                nc.sync.dma_start(out_r[sc, :, osl], ost[:])

            for it in range(NT):
                isl = slice(it * F, (it + 1) * F)
                fillers = (
                    [
                        (sc, ot)
                        for sc in range(4 * (it - 1), 4 * it)
                        for ot in range(NT)
                    ]
                    if it > 0
                    else []
                )
                fi = 0
                pair_idx = 0
                for h in range(LH):
                    po = acc_pool.tile([P, F], F32, tag="acc", name="po")
                    prs = []
                    qds = []
                    accl = None

                    def emit_avs(et, jj):
                        for e in range(2):
                            jc = 2 * jj + e
                            nc.tensor.matmul(
                                po[:],
                                lhsT=v_sb[:, jc, h * P : (h + 1) * P],
                                rhs=et[:, e, :],
                                start=(jc == 0),
                                stop=(jc == SC - 1),
                            )

                    # The av matmuls LAG their pair by 2: av(j) depends on
                    # exp(j), and issued back-to-back the PE would stall on
                    # the exp latency. Two pairs of scores + a wo group in
                    # between (~1.7us of independent PE work) cover it. The
                    # psum pair is freed by the exp itself (avs read the
                    # SBUF et tile), so this costs no extra PSUM banks.
                    pending = []
                    for jj in range(SC // 2):
                        ps = mm_pool.tile([P, 2, F], F32, tag="mm", name="pss")
                        for e in range(2):
                            jc = 2 * jj + e
                            # scores^T[j, i] = sum_hd k[hd, j] * q[hd, i]
                            nc.tensor.matmul(
                                ps[:, e, :],
                                lhsT=rot_sb[:, LH + h, jc * P : (jc + 1) * P],
                                rhs=rot_sb[:, h, isl],
                                start=True,
                                stop=True,
                            )
                        et = expp.tile([P, 2, F], CDT, tag="exp")
                        nc.scalar.activation(out=et[:], in_=ps[:], func=Exp)
                        if pair_idx % 2 == 1 and fi < len(fillers):
                            emit_wo_group(*fillers[fi])
                            fi += 1
                        pair_idx += 1
                        pending.append((et, jj))
                        if len(pending) > 2:
                            emit_avs(*pending.pop(0))
                        # Denominator tree on the DVE: bf16 pair/quad adds
                        # in the 2x mode, then f32 combines into accl.
                        pr = treep.tile([P, F], CDT, tag="pr")
                        nc.vector.tensor_add(pr[:], et[:, 0, :], et[:, 1, :])
                        prs.append(pr)
                        if len(prs) == 2:
                            qd = treep.tile([P, F], CDT, tag="qd")
                            nc.vector.tensor_add(qd[:], prs[0][:], prs[1][:])
                            prs = []
                            qds.append(qd)
                            if len(qds) == 2:
                                if accl is None:
                                    accl = small.tile(
                                        [P, F], mybir.dt.float32r, tag="accl"
                                    )
                                    nc.vector.tensor_add(
                                        accl[:], qds[0][:], qds[1][:]
                                    )
                                else:
                                    nc.vector.tensor_add(
                                        accl[:], accl[:], qds[0][:]
                                    )
                                    nc.vector.tensor_add(
                                        accl[:], accl[:], qds[1][:]
                                    )
                                qds = []
                    for pe_ in pending:
                        emit_avs(*pe_)
                    pl = l_pool.tile([P, F], F32, tag="lsum", name="pl")
                    nc.tensor.matmul(
                        pl[:],
                        lhsT=ones_fr[:],
                        rhs=accl[:],
                        start=True,
                        stop=True,
                    )
                    # pl rows are all equal (ones lhsT) -> reciprocal is
                    # already "broadcast" across partitions.
                    rl128 = small.tile([P, F], F32, tag="recip128")
                    nc.vector.reciprocal_approx_fast(rl128[:], pl[:])
                    nc.vector.tensor_mul(attn_sb[:, h, isl], po[:], rl128[:])
                assert fi == len(fillers)

            # wo tail: the last i-tile's output rows.
            for sc in range(4 * (NT - 1), 4 * NT):
                for ot in range(NT):
                    emit_wo_group(sc, ot)

    nc.compile()
    return nc


def shard_inputs(x, freqs_cis, wqkv, wo):
    """Produce the 8 per-core input maps (host-side layout prep)."""
    x = np.asarray(x, dtype=np.float32)
    freqs_cis = np.asarray(freqs_cis, dtype=np.float32)
    wqkv = np.asarray(wqkv, dtype=np.float32)
    wo = np.asarray(wo, dtype=np.float32)

    perm = np.concatenate([np.arange(0, HD, 2), np.arange(1, HD, 2)])  # even|odd
    cos = freqs_cis[:, :, 0].T  # [64, S]
    sin = freqs_cis[:, :, 1].T
    scale = 1.0 / np.sqrt(HD)  # folded into wq rows below
    tabc = np.concatenate([cos, cos], axis=0)  # [128, S]
    tabs = np.concatenate([-sin, sin], axis=0)  # sign baked in

    tabc = np.ascontiguousarray(tabc.astype(NP_CDT))
    tabs = np.ascontiguousarray(tabs.astype(NP_CDT))

    in_maps = []
    for c in range(N_CORES):
        b, g = divmod(c, GROUPS)
        heads = range(g * LH, (g + 1) * LH)
        wq_rows = np.concatenate(
            [wqkv[h * HD : (h + 1) * HD][perm] for h in heads], axis=0
        ) * scale  # [512, D]; 1/sqrt(hd) folded in
        wk_rows = np.concatenate(
            [wqkv[D + h * HD : D + (h + 1) * HD][perm] for h in heads], axis=0
        )
        wv_rows = np.concatenate(
            [wqkv[2 * D + h * HD : 2 * D + (h + 1) * HD] for h in heads], axis=0
        )
        wqk_l = np.concatenate([wq_rows, wk_rows], axis=0).T  # [D, 1024]
        wv_l = wv_rows.T  # [D, 512]
        din = np.concatenate([np.arange(h * HD, (h + 1) * HD) for h in heads])
        wo_l = wo[:, din].T  # [512, D]
        in_maps.append(
            {
                "xT": np.ascontiguousarray(x[b].T.astype(NP_CDT)),
                "wqk": np.ascontiguousarray(wqk_l.astype(NP_CDT)),
                "wv": np.ascontiguousarray(wv_l.astype(NP_CDT)),
                "wo": np.ascontiguousarray(wo_l.astype(NP_CDT)),
                "tabc": tabc,
                "tabs": tabs,
            }
        )
    return in_maps


def unshard_outputs(results):
    out = np.zeros((B, S, D), dtype=np.float32)
    for c in range(N_CORES):
        b = c // GROUPS
        out[b] += results[c]["out"].astype(np.float32)
    return out


_GRAPH_CACHE = {}


def kernel(x, freqs_cis, wqkv, wo):
    if "nc" not in _GRAPH_CACHE:
        _GRAPH_CACHE["nc"] = build_graph()
    nc = _GRAPH_CACHE["nc"]
    in_maps = shard_inputs(x, freqs_cis, wqkv, wo)
    res = run_bass_kernel_spmd(nc, in_maps, core_ids=list(range(N_CORES)))
    return unshard_outputs(res.results)
